# Initial kernel scaffold
#
"""Trainium2 Bass kernel for nn_MultiHeadAttention_73950746903066.

Reference computation (B=8, S=2048, D=1024, qdim=64):
    Qp = q @ Wq.T + bq;  Kp = k @ Wk.T + bk;  V = v @ Wv.T + bv
    Q2 = Qp @ Wq2.T + bq2;  K2 = Kp @ Wk2.T + bk2
    score = softmax(Q2 @ K2.T / sqrt(1024))
    out = score @ V

Algebraic simplifications (numerically validated to ~1e-7 rel err):
  * The chained projections collapse: Q2 = q @ (Wq2@Wq).T + (Wq2@bq + bq2),
    same for K2 -> the two [1024,1024] projections disappear.
  * Scores are tiny (max |score| ~ 1.5) so softmax needs no max-subtraction.
  * The K2 bias adds a per-row constant to scores -> cancels in softmax.
  * The Q2 bias contributes bqe.K2[t] which is folded in as a 65th
    contraction component (Q2a has a ones column, K2a a Bm.T@a column).
  * bv passes through the softmax-weighted average -> added on host.

Device layout (per core, data-parallel over batch, 1 batch element/core):
  Everything is computed with the contraction dim on SBUF partitions:
    Q2aT [128c, 2048s], K2aT [128c, 2048t]  (c padded 65->128 with zeros)
    V    [t, d] tiles, S_T [t, s] = K2aT.T @ Q2aT  (scores transposed!)
    E_T = exp(S_T/32);   rowsum via matmul with ones;  O[s,d] = E_T.T @ V
  so no on-chip transposes are needed anywhere.
"""

import os
import sys

import numpy as np

if "/opt/trn_rl_repo" not in sys.path:
    sys.path.insert(0, "/opt/trn_rl_repo")

B, S, D = 8, 2048, 1024
P = 128
NT = S // P  # 16 t-tiles
NSUP = S // 512  # 4 s-supers
N_CORES = 8

_CACHE = {}


def _build_program():
    import concourse.bass as bass
    import concourse.mybir as mybir
    import concourse.tile as tile
    from concourse import bacc

    f32 = mybir.dt.float32
    f32r = mybir.dt.float32r
    Exp = mybir.ActivationFunctionType.Exp

    nc = bacc.Bacc(
        "TRN2", target_bir_lowering=False, debug=False, num_devices=N_CORES
    )

    qT = nc.dram_tensor("qT", [D, S], f32, kind="ExternalInput").ap()
    kT = nc.dram_tensor("kT", [D, S], f32, kind="ExternalInput").ap()
    vT = nc.dram_tensor("vT", [D, S], f32, kind="ExternalInput").ap()
    CqT = nc.dram_tensor("CqT", [D, P], f32, kind="ExternalInput").ap()
    CkT = nc.dram_tensor("CkT", [D, P], f32, kind="ExternalInput").ap()
    WvT = nc.dram_tensor("WvT", [D, D], f32, kind="ExternalInput").ap()
    out = nc.dram_tensor("out", [S, D], f32, kind="ExternalOutput").ap()

    def r(x):
        return x.bitcast(f32r)

    with tile.TileContext(nc) as tc:
        from contextlib import ExitStack

        with ExitStack() as ctx:
            const_pool = ctx.enter_context(tc.tile_pool(name="const", bufs=1))
            vt_pool = ctx.enter_context(tc.tile_pool(name="vt", bufs=3))
            qk_pool = ctx.enter_context(tc.tile_pool(name="qk", bufs=3))
            big_pool = ctx.enter_context(tc.tile_pool(name="big", bufs=1))
            e_pool = ctx.enter_context(tc.tile_pool(name="e", bufs=32))
            on_pool = ctx.enter_context(tc.tile_pool(name="on", bufs=3))
            rc_pool = ctx.enter_context(tc.tile_pool(name="rc", bufs=2))
            ps_pool = ctx.enter_context(
                tc.tile_pool(name="ps", bufs=2, space="PSUM")
            )
            o_ps_pool = ctx.enter_context(
                tc.tile_pool(name="ops", bufs=4, space="PSUM")
            )
            rs_ps_pool = ctx.enter_context(
                tc.tile_pool(name="rsps", bufs=2, space="PSUM")
            )

            # ---- resident constants ----
            cq = const_pool.tile([P, D // P, P], f32, tag="cq")
            nc.sync.dma_start(cq[:], CqT.rearrange("(n p) c -> p n c", p=P))
            ck = const_pool.tile([P, D // P, P], f32, tag="ck")
            nc.sync.dma_start(ck[:], CkT.rearrange("(n p) c -> p n c", p=P))
            wv = const_pool.tile([P, D // P, D], f32, tag="wv")
            nc.sync.dma_start(wv[:], WvT.rearrange("(n p) c -> p n c", p=P))
            ones = const_pool.tile([P, 1], f32, tag="ones")
            nc.vector.memset(ones[:], 1.0)

            q2a = big_pool.tile([P, S], f32, tag="q2a")
            k2a = big_pool.tile([P, S], f32, tag="k2a")
            vsb = big_pool.tile([P, NT, D], f32, tag="v")

            # ---- Q2aT / K2aT projections: [128c, 2048] ----
            for which, (src, w, dst) in enumerate(
                [(qT, cq, q2a), (kT, ck, k2a)]
            ):
                for sc in range(NSUP):
                    xt = qk_pool.tile([P, D // P, 512], f32, tag="xt")
                    nc.sync.dma_start(
                        xt[:],
                        src[:, sc * 512 : (sc + 1) * 512].rearrange(
                            "(n p) m -> p n m", p=P
                        ),
                    )
                    ps = ps_pool.tile([P, 512], f32, tag="ps")
                    for d in range(D // P):
                        nc.tensor.matmul(
                            ps[:],
                            lhsT=r(w[:, d, :]),
                            rhs=r(xt[:, d, :]),
                            start=(d == 0),
                            stop=(d == D // P - 1),
                        )
                    nc.any.tensor_copy(dst[:, sc * 512 : (sc + 1) * 512], ps[:])
            # Q2a row 64 <- 1.0 (folds the q-side bias into the score matmul)
            nc.vector.memset(q2a[64:65, :], 1.0)

            # ---- V projection: V [t, d] tiles ----
            for t in range(NT):
                vt = vt_pool.tile([P, D // P, P], f32, tag="vt")
                nc.sync.dma_start(
                    vt[:],
                    vT[:, t * P : (t + 1) * P].rearrange("(n p) m -> p n m", p=P),
                )
                for dc in range(2):
                    ps = ps_pool.tile([P, 512], f32, tag="ps")
                    for e in range(D // P):
                        nc.tensor.matmul(
                            ps[:],
                            lhsT=r(vt[:, e, :]),
                            rhs=r(wv[:, e, dc * 512 : (dc + 1) * 512]),
                            start=(e == 0),
                            stop=(e == D // P - 1),
                        )
                    nc.any.tensor_copy(
                        vsb[:, t, dc * 512 : (dc + 1) * 512], ps[:]
                    )

            # ---- main attention loop over s-supers of 512 ----
            for sup in range(NSUP):
                e_tiles = []
                for t in range(NT):
                    st = ps_pool.tile([P, 512], f32, tag="ps")
                    nc.tensor.matmul(
                        st[:],
                        lhsT=r(k2a[:, t * P : (t + 1) * P]),
                        rhs=r(q2a[:, sup * 512 : (sup + 1) * 512]),
                        start=True,
                        stop=True,
                    )
                    et = e_pool.tile([P, 512], f32, tag="et")
                    nc.scalar.activation(et[:], st[:], Exp, scale=1.0 / 32.0)
                    e_tiles.append(et)

                for si in range(4):
                    o0 = o_ps_pool.tile([P, 512], f32, tag="ops")
                    o1 = o_ps_pool.tile([P, 512], f32, tag="ops")
                    rs = rs_ps_pool.tile([P, 1], f32, tag="rsps")
                    for t in range(NT):
                        stat = r(e_tiles[t][:, si * P : (si + 1) * P])
                        nc.tensor.matmul(
                            o0[:], lhsT=stat, rhs=r(vsb[:, t, 0:512]),
                            start=(t == 0), stop=(t == NT - 1),
                        )
                        nc.tensor.matmul(
                            o1[:], lhsT=stat, rhs=r(vsb[:, t, 512:1024]),
                            start=(t == 0), stop=(t == NT - 1),
                        )
                        nc.tensor.matmul(
                            rs[:], lhsT=stat, rhs=r(ones[:]),
                            start=(t == 0), stop=(t == NT - 1),
                        )
                    rc = rc_pool.tile([P, 1], f32, tag="rc")
                    nc.vector.reciprocal(rc[:], rs[:])
                    on = on_pool.tile([P, D], f32, tag="on")
                    nc.vector.tensor_scalar_mul(on[:, 0:512], o0[:], rc[:])
                    nc.vector.tensor_scalar_mul(on[:, 512:1024], o1[:], rc[:])
                    srow = (sup * 4 + si) * P
                    nc.sync.dma_start(out[srow : srow + P, :], on[:])

    nc.compile()
    return nc


def _get_program():
    if "nc" not in _CACHE:
        _CACHE["nc"] = _build_program()
    return _CACHE["nc"]


def _prep_inputs(q, k, v, Wq, bq, Wk, bk, Wv, bv, Wq2, bq2, Wk2, bk2):
    """Host-side weight folding + per-batch activation transposes."""
    f32 = np.float32
    A = (Wq2 @ Wq).astype(f32)  # [64, 1024]
    a = (Wq2 @ bq + bq2).astype(f32)  # [64]
    Bm = (Wk2 @ Wk).astype(f32)  # [64, 1024]

    CqT = np.zeros((D, P), f32)
    CqT[:, :64] = A.T
    CkT = np.zeros((D, P), f32)
    CkT[:, :64] = Bm.T
    CkT[:, 64] = Bm.T @ a
    WvT = np.ascontiguousarray(Wv.T).astype(f32)

    in_maps = []
    for b in range(B):
        in_maps.append(
            {
                "qT": np.ascontiguousarray(q[b].T, dtype=f32),
                "kT": np.ascontiguousarray(k[b].T, dtype=f32),
                "vT": np.ascontiguousarray(v[b].T, dtype=f32),
                "CqT": CqT,
                "CkT": CkT,
                "WvT": WvT,
            }
        )
    return in_maps


def kernel(q, k, v, Wq, bq, Wk, bk, Wv, bv, Wq2, bq2, Wk2, bk2, _debug=None):
    from concourse.bass_utils import run_bass_kernel_spmd

    nc = _get_program()
    in_maps = _prep_inputs(q, k, v, Wq, bq, Wk, bk, Wv, bv, Wq2, bq2, Wk2, bk2)

    kwargs = dict(_debug or {})
    res = run_bass_kernel_spmd(nc, in_maps, core_ids=list(range(N_CORES)), **kwargs)
    if _debug is not None:
        _CACHE["last_result"] = res

    outs = np.stack([res.results[b]["out"] for b in range(B)])
    return (outs + bv.astype(np.float32)).astype(np.float32)


# revision 15
# speedup vs baseline: 1.5942x; 1.5942x over previous
"""Trainium2 Bass kernel for nn_MultiHeadAttention_73950746903066.

Reference computation (B=8, S=2048, D=1024, qdim=64):
    Qp = q @ Wq.T + bq;  Kp = k @ Wk.T + bk;  V = v @ Wv.T + bv
    Q2 = Qp @ Wq2.T + bq2;  K2 = Kp @ Wk2.T + bk2
    score = softmax(Q2 @ K2.T / sqrt(1024))
    out = score @ V

Algebraic simplifications (numerically validated to ~1e-7 rel err):
  * The chained projections collapse: Q2 = q @ (Wq2@Wq).T + (Wq2@bq + bq2),
    same for K2 -> the two [1024,1024] projections disappear.
  * Scores are tiny (max |score| ~ 1.5) so softmax needs no max-subtraction.
  * The K2 bias adds a per-row constant to scores -> cancels in softmax.
  * The Q2 bias contributes bqe.K2[t] which is folded in as a 65th
    contraction component (Q2a has a ones column, K2a a Bm.T@a column).
  * bv passes through the softmax-weighted average -> added on host.

Device layout (per core, data-parallel over batch, 1 batch element/core):
  Everything is computed with the contraction dim on SBUF partitions:
    Q2aT [128c, 2048s], K2aT [128c, 2048t]  (c padded 65->128 with zeros)
    V    [t, d] tiles, S_T [t, s] = K2aT.T @ Q2aT  (scores transposed!)
    E_T = exp(S_T/32);   rowsum via matmul with ones;  O[s,d] = E_T.T @ V
  so no on-chip transposes are needed anywhere.

All matmuls run in float32r (TF32-like: 8-bit exp, 11-bit mantissa) which
streams at bf16 rate on the PE array while keeping ~1e-4 precision.
"""

import os
import sys

import numpy as np

if "/opt/trn_rl_repo" not in sys.path:
    sys.path.insert(0, "/opt/trn_rl_repo")

B, S, D = 8, 2048, 1024
P = 128
NT = S // P  # 16 t-tiles
SUPW = 256  # s-super width (score-matmul free dim; >=256 keeps f32r fast)
NSUP = S // SUPW
N_CORES = 8

_CACHE = {}


def _build_program(bench_reps=1):
    import concourse.mybir as mybir
    import concourse.tile as tile
    from concourse import bacc
    from contextlib import ExitStack

    f32 = mybir.dt.float32
    f32r = mybir.dt.float32r
    Exp = mybir.ActivationFunctionType.Exp

    nc = bacc.Bacc(
        "TRN2", target_bir_lowering=False, debug=False, num_devices=N_CORES
    )

    qT = nc.dram_tensor("qT", [D, S], f32r, kind="ExternalInput").ap()
    kT = nc.dram_tensor("kT", [D, S], f32r, kind="ExternalInput").ap()
    vT = nc.dram_tensor("vT", [D, S], f32r, kind="ExternalInput").ap()
    CqT = nc.dram_tensor("CqT", [D, P], f32r, kind="ExternalInput").ap()
    CkT = nc.dram_tensor("CkT", [D, P], f32r, kind="ExternalInput").ap()
    WvT = nc.dram_tensor("WvT", [D, D], f32r, kind="ExternalInput").ap()
    out = nc.dram_tensor("out", [S, D], f32, kind="ExternalOutput").ap()

    with tile.TileContext(nc) as tc:
        with ExitStack() as ctx:
            const_pool = ctx.enter_context(tc.tile_pool(name="const", bufs=1))
            vt_pool = ctx.enter_context(tc.tile_pool(name="vt", bufs=3))
            qk_pool = ctx.enter_context(tc.tile_pool(name="qk", bufs=10))
            big_pool = ctx.enter_context(tc.tile_pool(name="big", bufs=1))
            e_pool = ctx.enter_context(tc.tile_pool(name="e", bufs=24))
            on_pool = ctx.enter_context(tc.tile_pool(name="on", bufs=3))
            rc_pool = ctx.enter_context(tc.tile_pool(name="rc", bufs=2))
            ps_pool = ctx.enter_context(
                tc.tile_pool(name="ps", bufs=2, space="PSUM")
            )
            o_ps_pool = ctx.enter_context(
                tc.tile_pool(name="ops", bufs=4, space="PSUM")
            )
            rs_ps_pool = ctx.enter_context(
                tc.tile_pool(name="rsps", bufs=2, space="PSUM")
            )

            # ---- resident constants (outside the bench loop) ----
            cq = const_pool.tile([P, D // P, P], f32r, tag="cq")
            nc.sync.dma_start(cq[:], CqT.rearrange("(n p) c -> p n c", p=P))
            ck = const_pool.tile([P, D // P, P], f32r, tag="ck")
            nc.sync.dma_start(ck[:], CkT.rearrange("(n p) c -> p n c", p=P))
            wv = const_pool.tile([P, D // P, D], f32r, tag="wv")
            nc.sync.dma_start(wv[:], WvT.rearrange("(n p) c -> p n c", p=P))
            ones_f32 = const_pool.tile([P, 2], f32, tag="ones_f32")
            nc.vector.memset(ones_f32[:], 1.0)
            ones = const_pool.tile([P, 2], f32r, tag="ones")
            nc.vector.tensor_copy(ones[:], ones_f32[:])
            ones_row = const_pool.tile([1, S], f32, tag="ones_row")
            nc.vector.memset(ones_row[:], 1.0)

            def body(_iv=None):
                q2a = big_pool.tile([P, S], f32r, tag="q2a")
                k2a = big_pool.tile([P, S], f32r, tag="k2a")
                vsb = big_pool.tile([P, NT, D], f32r, tag="v")

                # ---- Q2aT / K2aT projections: [128c, 2048] ----
                for src, w, dst in [(qT, cq, q2a), (kT, ck, k2a)]:
                    for sc in range(S // 512):
                        xts = []
                        for d in range(D // P):
                            xt = qk_pool.tile([P, 512], f32r, tag="xt")
                            nc.sync.dma_start(
                                xt[:],
                                src[
                                    d * P : (d + 1) * P,
                                    sc * 512 : (sc + 1) * 512,
                                ],
                            )
                            xts.append(xt)
                        ps = ps_pool.tile([P, 512], f32, tag="ps")
                        for d in range(D // P):
                            nc.tensor.matmul(
                                ps[:],
                                lhsT=w[:, d, :],
                                rhs=xts[d][:],
                                start=(d == 0),
                                stop=(d == D // P - 1),
                            )
                        nc.any.tensor_copy(
                            dst[:, sc * 512 : (sc + 1) * 512], ps[:]
                        )
                # Q2a row 64 <- 1.0 (folds the q-side bias into the scores)
                nc.vector.tensor_copy(q2a[64:65, :], ones_row[:])

                # ---- V projection: V [t, d] tiles ----
                for t in range(NT):
                    vt = vt_pool.tile([P, D // P, P], f32r, tag="vt")
                    nc.sync.dma_start(
                        vt[:],
                        vT[:, t * P : (t + 1) * P].rearrange(
                            "(n p) m -> p n m", p=P
                        ),
                    )
                    for dc in range(2):
                        ps = ps_pool.tile([P, 512], f32, tag="ps")
                        for e in range(D // P):
                            nc.tensor.matmul(
                                ps[:],
                                lhsT=vt[:, e, :],
                                rhs=wv[:, e, dc * 512 : (dc + 1) * 512],
                                start=(e == 0),
                                stop=(e == D // P - 1),
                            )
                        nc.any.tensor_copy(
                            vsb[:, t, dc * 512 : (dc + 1) * 512], ps[:]
                        )

                # ---- main attention loop over s-supers ----
                for sup in range(NSUP):
                    e_tiles = []
                    for t in range(NT):
                        st = ps_pool.tile([P, SUPW], f32, tag="ps")
                        nc.tensor.matmul(
                            st[:],
                            lhsT=k2a[:, t * P : (t + 1) * P],
                            rhs=q2a[:, sup * SUPW : (sup + 1) * SUPW],
                            start=True,
                            stop=True,
                        )
                        et = e_pool.tile([P, SUPW], f32r, tag="et")
                        nc.scalar.activation(
                            et[:], st[:], Exp, scale=1.0 / 32.0
                        )
                        e_tiles.append(et)

                    for si in range(SUPW // P):
                        o0 = o_ps_pool.tile([P, 512], f32, tag="ops")
                        o1 = o_ps_pool.tile([P, 512], f32, tag="ops")
                        rs = rs_ps_pool.tile([P, 2], f32, tag="rsps")
                        for t in range(NT):
                            stat = e_tiles[t][:, si * P : (si + 1) * P]
                            nc.tensor.matmul(
                                o0[:], lhsT=stat, rhs=vsb[:, t, 0:512],
                                start=(t == 0), stop=(t == NT - 1),
                            )
                            nc.tensor.matmul(
                                o1[:], lhsT=stat, rhs=vsb[:, t, 512:1024],
                                start=(t == 0), stop=(t == NT - 1),
                            )
                            nc.tensor.matmul(
                                rs[:], lhsT=stat, rhs=ones[:],
                                start=(t == 0), stop=(t == NT - 1),
                            )
                        rc = rc_pool.tile([P, 1], f32, tag="rc")
                        nc.vector.reciprocal(rc[:], rs[:, 0:1])
                        on = on_pool.tile([P, D], f32, tag="on")
                        nc.vector.tensor_scalar_mul(on[:, 0:512], o0[:], rc[:])
                        nc.vector.tensor_scalar_mul(
                            on[:, 512:1024], o1[:], rc[:]
                        )
                        srow = sup * SUPW + si * P
                        nc.sync.dma_start(out[srow : srow + P, :], on[:])

            if bench_reps > 1:
                with tc.For_i(0, bench_reps, 1) as iv:
                    body(iv)
            else:
                body()

    nc.compile()
    return nc


def _get_program():
    if "nc" not in _CACHE:
        _CACHE["nc"] = _build_program()
    return _CACHE["nc"]


def _round_f32r(x):
    """Round fp32 to the fp32r grid (sign + 8-bit exp + 11-bit mantissa),
    round-to-nearest-even on the dropped 12 bits — matches walrus's
    fp32_to_fp32r so host-side values equal what the PE array consumes."""
    u = np.ascontiguousarray(x, np.float32).view(np.uint32)
    low = u & np.uint32(0xFFF)
    base = u & np.uint32(0xFFFFF000)
    lsb = (u >> np.uint32(12)) & np.uint32(1)
    round_up = (low > 0x800) | ((low == 0x800) & (lsb == 1))
    out = base + (round_up.astype(np.uint32) << np.uint32(12))
    return out.view(np.float32)


def _prep_inputs(q, k, v, Wq, bq, Wk, bk, Wv, bv, Wq2, bq2, Wk2, bk2):
    """Host-side weight folding + per-batch activation transposes."""
    f32 = np.float32
    A = (Wq2 @ Wq).astype(f32)  # [64, 1024]
    a = (Wq2 @ bq + bq2).astype(f32)  # [64]
    Bm = (Wk2 @ Wk).astype(f32)  # [64, 1024]

    CqT = np.zeros((D, P), f32)
    CqT[:, :64] = A.T
    CkT = np.zeros((D, P), f32)
    CkT[:, :64] = Bm.T
    CkT[:, 64] = Bm.T @ a
    WvT = np.ascontiguousarray(Wv.T).astype(f32)

    CqT = _round_f32r(CqT)
    CkT = _round_f32r(CkT)
    WvT = _round_f32r(WvT)
    in_maps = []
    for b in range(B):
        in_maps.append(
            {
                "qT": _round_f32r(q[b].T),
                "kT": _round_f32r(k[b].T),
                "vT": _round_f32r(v[b].T),
                "CqT": CqT,
                "CkT": CkT,
                "WvT": WvT,
            }
        )
    return in_maps


def kernel(q, k, v, Wq, bq, Wk, bk, Wv, bv, Wq2, bq2, Wk2, bk2, _debug=None):
    from concourse.bass_utils import run_bass_kernel_spmd

    nc = _get_program()
    in_maps = _prep_inputs(q, k, v, Wq, bq, Wk, bk, Wv, bv, Wq2, bq2, Wk2, bk2)

    kwargs = dict(_debug or {})
    res = run_bass_kernel_spmd(nc, in_maps, core_ids=list(range(N_CORES)), **kwargs)
    if _debug is not None:
        _CACHE["last_result"] = res

    outs = np.stack([res.results[b]["out"] for b in range(B)])
    return (outs + bv.astype(np.float32)).astype(np.float32)
